# revision 38
# baseline (speedup 1.0000x reference)
"""GPT2-style fused causal attention (DecisionTransformer) on 8 Trainium2
NeuronCores — v4.

Sharding: tensor-parallel over the 16 heads (2 heads / core, both batches on
every core), row-parallel output projection; host sums the 8 partials.

vs v3 (188 us):
  - The exp conveyor starts at ~7 us instead of ~40 us: QKV runs as
    quarter-waves (1 PSUM bank, 512 tokens) and attention for (b0, qc0)
    begins as soon as the first q/k/v quarter + 4 vaug blocks exist.
    All remaining QKV waves, vaug builds and projections are emitted as
    filler units inside the attention kb loops.
  - V transposes packed: one [128,128] PE transpose per key block (both
    heads at once) instead of two [64,128] ones; vaug is a single
    [P, NKB, 2, P] tile per batch written with one strided DVE copy.
  - Causal masks run on the (otherwise idle) GpSimd engine.
  - Normalize: reciprocal on the [1, QC] denominator row BEFORE the
    partition broadcast (recip work drops 64x), broadcasts on GpSimd.
  - Evictions alternate between scalar/vector engines to balance load.
"""

import sys

for _p in ("/opt/trn_rl_repo",):
    if _p not in sys.path:
        sys.path.insert(0, _p)

import numpy as np
import ml_dtypes

import concourse.bass as bass
import concourse.mybir as mybir
import concourse.tile as tile
from concourse import bacc
from concourse.bass_utils import run_bass_kernel_spmd

P = 128
B, S, D, H, HD = 2, 2048, 1024, 16, 64
T = B * S              # 4096 tokens
KO = D // P            # 8 contraction chunks
QC = 512               # query chunk
NQC = S // QC          # 4
NKB = S // P           # 16 key blocks per sequence
SCALE = 1.0 / float(HD) ** 0.5
N_CORES = 8
HPC = H // N_CORES     # 2 heads per core

f32 = mybir.dt.float32
f32r = mybir.dt.float32r
bf16 = mybir.dt.bfloat16
BF = ml_dtypes.bfloat16


def _build_program():
    nc = bacc.Bacc(None, target_bir_lowering=False)

    xt_d = nc.dram_tensor("xt", [D, T], bf16, kind="ExternalInput")
    wqkv_d = nc.dram_tensor("w_qkv", [P, 3, KO * P], bf16, kind="ExternalInput")
    bqkv_d = nc.dram_tensor("b_qkv", [P, 3], f32, kind="ExternalInput")
    wp_d = nc.dram_tensor("w_proj", [P, D], bf16, kind="ExternalInput")
    # rank-127 factorization of the strict upper triangle: mu.T @ mv =
    # -1e5 * (k > q); accumulated onto diagonal score blocks in PSUM so
    # exp underflows masked entries to exactly 0 (no elementwise mask op)
    mu_d = nc.dram_tensor("mu", [P - 1, P], bf16, kind="ExternalInput")
    mv_d = nc.dram_tensor("mv", [P - 1, P], bf16, kind="ExternalInput")
    id128_d = nc.dram_tensor("id128", [P, P], bf16, kind="ExternalInput")
    out_d = nc.dram_tensor("out", [T, D], bf16, kind="ExternalOutput")

    with tile.TileContext(nc) as tc:
        with (
            tc.tile_pool(name="const", bufs=1) as const,
            tc.tile_pool(name="pt", bufs=5) as pt_pool,
            tc.tile_pool(name="atn", bufs=4) as atn_pool,
            tc.tile_pool(name="den", bufs=4) as den_pool,
            tc.tile_pool(name="rbs", bufs=4) as rbs_pool,
            tc.tile_pool(name="ot", bufs=3) as ot_pool,
            tc.tile_pool(name="ps_a", bufs=2, space="PSUM") as ps_a,
            tc.tile_pool(name="ps_sc", bufs=3, space="PSUM") as ps_sc,
            tc.tile_pool(name="ps_po", bufs=3, space="PSUM") as ps_po,
        ):
            # ---- constants (all host-prepared) ----
            bqkv_sb = const.tile([P, 3], f32)
            nc.sync.dma_start(bqkv_sb[:], bqkv_d[:])
            id128_sb = const.tile([P, P], bf16)
            nc.sync.dma_start(id128_sb[:], id128_d[:])
            # wqkv is fc-major [p, fc, ko, i]: per-fc slices are contiguous
            # so the first q columns land early in the DMA stream
            wqkv_sb = const.tile([P, 3, KO * P], bf16)
            nc.sync.dma_start(wqkv_sb[:, 0, :], wqkv_d[:, 0, :])
            mu_sb = const.tile([P - 1, P], bf16)
            nc.scalar.dma_start(mu_sb[:], mu_d[:])
            mv_sb = const.tile([P - 1, P], bf16)
            nc.scalar.dma_start(mv_sb[:], mv_d[:])
            wp_sb = const.tile([P, D], bf16)

            # persistent SBUF state (split per batch / per ko so the tile
            # dependency tracking stays fine-grained)
            xts = [
                [const.tile([P, S], bf16, name=f"xts{b}_{ko}") for ko in range(KO)]
                for b in range(B)
            ]
            # zero-padded Q^T per (batch, head): the other head's 64 rows
            # stay zero so full-128-contraction scores matmuls are exact
            qpad = [
                [const.tile([P, S], bf16, name=f"qp{b}{h}") for h in range(HPC)]
                for b in range(B)
            ]
            kT = [const.tile([P, S], bf16, name=f"kT{b}") for b in range(B)]
            vT = [const.tile([P, S], bf16, name=f"vT{b}") for b in range(B)]
            # V natural layout per batch: [keys, kb, head, 64 V | ones | 0pad]
            # (128-wide lhsT keeps the AV matmuls registering in the PE
            # clock gate; ones column yields the softmax denominator)
            vaug = [
                const.tile([P, NKB, HPC, P], bf16, name=f"vaug{b}")
                for b in range(B)
            ]
            # constant pad regions on the (idle-at-start) gpsimd engine so
            # neither the DMA rings nor the vector engine pay for them
            for b in range(B):
                nc.gpsimd.memset(qpad[b][0][HD:, :], 0.0)
                nc.gpsimd.memset(qpad[b][1][:HD, :], 0.0)
                nc.gpsimd.memset(vaug[b][:, :, :, HD : HD + 1], 1.0)
                nc.gpsimd.memset(vaug[b][:, :, :, HD + 1 :], 0.0)

            # HAM warmup: matmul activity while the first X chunks stream in
            for w in range(20):
                psw = ps_sc.tile([P, QC], f32, tag="sc", name="psw")
                nc.tensor.matmul(
                    psw[:, :P], id128_sb[:], id128_sb[:], start=True, stop=True
                )

            # X^T loads: batch 0 in 512-token x ko chunks so the first
            # quarter-wave completes after ~1 MB; batch 1 as full rows
            # X loads alternate between the sync and scalar DGE rings for
            # ~2x aggregate DMA bandwidth during the prologue
            def xload(eng, dst, src):
                (nc.sync if eng == 0 else nc.scalar).dma_start(dst, src)

            for ko in range(KO):
                xload(ko % 2, xts[0][ko][:, 0:QC],
                      xt_d[ko * P : (ko + 1) * P, 0:QC])
            nc.sync.dma_start(wqkv_sb[:, 1, :], wqkv_d[:, 1, :])
            nc.scalar.dma_start(wqkv_sb[:, 2, :], wqkv_d[:, 2, :])
            nc.sync.dma_start(wp_sb[:], wp_d[:])
            for tq in range(1, 4):
                for ko in range(KO):
                    xload(ko % 2, xts[0][ko][:, tq * QC : (tq + 1) * QC],
                          xt_d[ko * P : (ko + 1) * P, tq * QC : (tq + 1) * QC])
            for ko in range(KO):
                xload(ko % 2, xts[1][ko][:],
                      xt_d[ko * P : (ko + 1) * P, S : 2 * S])

            # ---------------- emit helpers ----------------
            ecnt = [0]

            def evict(dst_ap, src_ap, bias_ap=None, on_act=False):
                """PSUM->SBUF eviction. Once the exp conveyor is running,
                the scalar queue is deep — only route there when asked."""
                if bias_ap is None:
                    if on_act:
                        nc.scalar.copy(dst_ap, src_ap)
                    else:
                        nc.vector.tensor_copy(dst_ap, src_ap)
                elif on_act:
                    nc.scalar.activation(
                        dst_ap, src_ap,
                        mybir.ActivationFunctionType.Identity, bias=bias_ap,
                    )
                else:
                    nc.vector.tensor_scalar(
                        dst_ap, src_ap, bias_ap, None, mybir.AluOpType.add
                    )

            def qkv_half(b, fc, tq, half, state, on_act=False):
                """Half of a quarter wave: 4 ko-accumulation matmuls; the
                second half evicts. Splitting keeps PE filler bursts short
                so the score matmuls (and the exp conveyor) never starve."""
                if half == 0:
                    state["ps"] = ps_a.tile(
                        [P, QC], f32, tag="a", name=f"qkv{b}{fc}{tq}"
                    )
                ps = state["ps"]
                for ko in range(half * 4, half * 4 + 4):
                    nc.tensor.matmul(
                        ps[:],
                        wqkv_sb[:, fc, ko * P : (ko + 1) * P],
                        xts[b][ko][:, tq * QC : (tq + 1) * QC],
                        start=(ko == 0),
                        stop=(ko == KO - 1),
                    )
                if half == 0:
                    return
                cs = slice(tq * QC, (tq + 1) * QC)
                if fc == 0:
                    evict(qpad[b][0][:HD, cs], ps[:HD], bqkv_sb[:HD, 0:1],
                          on_act=on_act)
                    evict(qpad[b][1][HD:, cs], ps[HD:], bqkv_sb[HD:, 0:1],
                          on_act=on_act)
                else:
                    dst = kT[b] if fc == 1 else vT[b]
                    evict(dst[:, cs], ps[:], bqkv_sb[:, fc : fc + 1],
                          on_act=on_act)

            def qkv_wave(b, fc, tq, on_act=False):
                state = {}
                qkv_half(b, fc, tq, 0, state, on_act)
                qkv_half(b, fc, tq, 1, state, on_act)

            def vaug1(b, kb):
                """V natural layout for one key block: a single [128,128]
                PE transpose (both heads), one strided DVE copy."""
                psT = ps_a.tile([P, HPC, HD], bf16, tag="a", name=f"va{b}{kb}")
                nc.tensor.transpose(
                    psT[:], vT[b][:, kb * P : (kb + 1) * P], id128_sb[:]
                )
                nc.vector.tensor_copy(vaug[b][:, kb, :, :HD], psT[:])

            atn = [[None] * NQC for _ in range(B)]

            def attn_qc(b, qc, filler, draws=1):
                """Causal attention for both heads of batch b, query chunk
                qc; `draws` filler units are drained per key block."""
                nkb = (qc + 1) * (QC // P)
                po = [
                    ps_po.tile([P, QC], f32, tag="po", name=f"po{b}{qc}{h}")
                    for h in range(HPC)
                ]

                def av_pair(kb, pts, lo):
                    for hl in range(HPC):
                        nc.tensor.matmul(
                            po[hl][:, lo:],
                            vaug[b][:, kb, hl, :],
                            pts[hl][:, lo:],
                            start=(kb == 0),
                            stop=(kb == nkb - 1),
                        )

                prev = None
                for kb in range(nkb):
                    j = kb - qc * (QC // P)
                    lo = j * P if j > 0 else 0
                    pts = []
                    for hl in range(HPC):
                        sc = ps_sc.tile([P, QC], f32, tag="sc", name=f"sc{hl}")
                        nc.tensor.matmul(
                            sc[:, lo:],
                            kT[b][:, kb * P : (kb + 1) * P],
                            qpad[b][hl][:, qc * QC + lo : (qc + 1) * QC],
                            start=True,
                            stop=(j < 0),
                        )
                        if j >= 0:
                            # fold the causal mask into the score block:
                            # += mu.T @ mv = -1e5 above the diagonal
                            js = slice(j * P, (j + 1) * P)
                            nc.tensor.matmul(
                                sc[:, js], mu_sb[:], mv_sb[:],
                                start=False, stop=True,
                            )
                        pt = pt_pool.tile([P, QC], bf16, tag="pt", name=f"pt{hl}")
                        nc.scalar.activation(
                            pt[:, lo:], sc[:, lo:],
                            mybir.ActivationFunctionType.Exp, scale=SCALE,
                        )
                        pts.append(pt)
                    # AV lags the scores by one key block so the exp
                    # conveyor always has the next block queued
                    if prev is not None:
                        av_pair(*prev)
                    prev = (kb, pts, lo)
                    for _ in range(draws):
                        u = next(filler, None)
                        if u is not None:
                            u()
                av_pair(*prev)
                # normalize: den row -> reciprocal -> gpsimd broadcast ->
                # one DVE mult per head
                rbs = []
                for hl in range(HPC):
                    den = den_pool.tile([1, QC], f32, tag="den", name=f"dn{hl}")
                    if hl == 0:
                        nc.vector.tensor_copy(den[:], po[hl][HD : HD + 1, :])
                    else:
                        nc.scalar.copy(den[:], po[hl][HD : HD + 1, :])
                    rcp = den_pool.tile([1, QC], f32, tag="den", name=f"rc{hl}")
                    nc.vector.reciprocal_approx_fast(out=rcp[:], in_=den[:])
                    rb = rbs_pool.tile([HD, QC], f32, tag="rb", name=f"rb{hl}")
                    nc.gpsimd.partition_broadcast(rb[:], rcp[:], channels=HD)
                    rbs.append(rb)
                at = atn_pool.tile([P, QC], bf16, tag="atn", name=f"atn{b}{qc}")
                for hl in range(HPC):
                    hp = slice(hl * HD, (hl + 1) * HD)
                    nc.vector.tensor_tensor(
                        at[hp, :], po[hl][:HD, :], rbs[hl][:],
                        mybir.AluOpType.mult,
                    )
                atn[b][qc] = at

            def proj_unit(b, qc, qb, split_dma=False):
                ot = ot_pool.tile([P, D], bf16, tag="ot", name="ot")
                for nck in range(2):
                    pp = ps_a.tile(
                        [P, D // 2], f32, tag="a", name=f"pp{b}{qc}{qb}{nck}"
                    )
                    nc.tensor.matmul(
                        pp[:],
                        atn[b][qc][:, qb * P : (qb + 1) * P],
                        wp_sb[:, nck * (D // 2) : (nck + 1) * (D // 2)],
                        start=True,
                        stop=True,
                    )
                    on_act = ecnt[0] % 4 == 0
                    ecnt[0] += 1
                    evict(ot[:, nck * (D // 2) : (nck + 1) * (D // 2)], pp[:],
                          on_act=on_act)
                row = b * S + qc * QC + qb * P
                if split_dma:
                    nc.sync.dma_start(
                        out_d[row : row + P, : D // 2], ot[:, : D // 2]
                    )
                    nc.sync.dma_start(
                        out_d[row : row + P, D // 2 :], ot[:, D // 2 :]
                    )
                else:
                    nc.sync.dma_start(out_d[row : row + P, :], ot[:])

            # ---------------- schedule ----------------
            # prefix: first q/k/v quarters of batch 0 + vaug for kb 0-3
            # (scalar engine is still idle here — evict there)
            for fc in range(3):
                qkv_wave(0, fc, 0, on_act=True)
            for kb in range(4):
                vaug1(0, kb)

            def units():
                # (b, tq) quarter batches in need order: b0 tq1-3, b1 tq0-3.
                # Each is 6 wave half-units + 4 vaug units = 10 units; all 70
                # drain during b0 attention (2 units per key block).
                for b, tq in [(0, 1), (0, 2), (0, 3),
                              (1, 0), (1, 1), (1, 2), (1, 3)]:
                    for fc in range(3):
                        st = {}
                        yield lambda b=b, fc=fc, tq=tq, st=st: qkv_half(
                            b, fc, tq, 0, st)
                        yield lambda b=b, fc=fc, tq=tq, st=st: qkv_half(
                            b, fc, tq, 1, st)
                    for kb in range(4 * tq, 4 * tq + 4):
                        yield lambda b=b, kb=kb: vaug1(b, kb)
                # projections: consumed at the tail of b0 attention (draws
                # 71-80) and through b1 attention (1 per key block, draws
                # 81-120); bubbles delay proj(1,x) past attn(1,x)'s end
                for pb, pqc in [(0, 0), (0, 1), (0, 2), (0, 3)]:
                    for qb in range(QC // P):
                        yield lambda pb=pb, pqc=pqc, qb=qb: proj_unit(pb, pqc, qb)
                for qb in range(QC // P):          # draws 87-90 (attn(1,1))
                    yield lambda qb=qb: proj_unit(1, 0, qb)
                yield None
                yield None
                for qb in range(QC // P):          # draws 93-96 (attn(1,2))
                    yield lambda qb=qb: proj_unit(1, 1, qb)
                for _ in range(8):
                    yield None
                for qb in range(QC // P):          # draws 105-108 (attn(1,3))
                    yield lambda qb=qb: proj_unit(1, 2, qb)

            filler = units()
            for qc in range(NQC):
                attn_qc(0, qc, filler, draws=2)
            for qc in range(NQC):
                attn_qc(1, qc, filler, draws=1)
            # drain any unconsumed filler units (projections)
            for u in filler:
                if u is not None:
                    u()
            for qb in range(QC // P):
                proj_unit(1, 3, qb, split_dma=True)

    nc.compile()
    return nc


_CACHE = {}


def get_program():
    if "nc" not in _CACHE:
        _CACHE["nc"] = _build_program()
    return _CACHE["nc"]


def make_in_maps(hidden_states, c_attn_w, c_attn_b, c_proj_w):
    x = np.asarray(hidden_states, dtype=np.float32).reshape(T, D)
    xt = np.ascontiguousarray(x.T).astype(BF)                     # [D, T]
    wa = np.asarray(c_attn_w, dtype=np.float32)
    ba = np.asarray(c_attn_b, dtype=np.float32)
    wp = np.asarray(c_proj_w, dtype=np.float32)

    # mu.T @ mv = -1e5 * strict_upper_triangle(P): mu[c,k] = (k > c),
    # mv[c,q] = -1e5 * (q == c)
    cc, kk = np.meshgrid(np.arange(P - 1), np.arange(P), indexing="ij")
    mu = (kk > cc).astype(BF)                                     # [P-1, P]
    mv = np.zeros((P - 1, P), dtype=np.float32)
    mv[np.arange(P - 1), np.arange(P - 1)] = -1e5
    mv = mv.astype(BF)
    id128 = np.eye(P, dtype=np.float32).astype(BF)                # [P, P]

    in_maps = []
    for core in range(N_CORES):
        lo = core * P
        # [d, fc, i] -> [p, ko, fc, i] -> [P, KO*3*P]
        wa3 = np.stack(
            [wa[:, lo : lo + P], wa[:, D + lo : D + lo + P],
             wa[:, 2 * D + lo : 2 * D + lo + P]],
            axis=1,
        )                                                          # [D, 3, P]
        # fc-major: [p, fc, ko, i]
        wq = np.ascontiguousarray(
            wa3.reshape(KO, P, 3, P).transpose(1, 2, 0, 3).reshape(P, 3, KO * P)
        ).astype(BF)
        bq = np.ascontiguousarray(
            np.stack(
                [ba[lo : lo + P], ba[D + lo : D + lo + P],
                 ba[2 * D + lo : 2 * D + lo + P]],
                axis=1,
            )
        ).astype(np.float32)                                       # [P, 3]
        wpc = np.ascontiguousarray(wp[lo : lo + P, :]).astype(BF)  # [P, D]
        in_maps.append(
            {
                "xt": xt,
                "w_qkv": wq,
                "b_qkv": bq,
                "w_proj": wpc,
                "mu": mu,
                "mv": mv,
                "id128": id128,
            }
        )
    return in_maps


def kernel(hidden_states, c_attn_w, c_attn_b, c_proj_w, c_proj_b):
    nc = get_program()
    in_maps = make_in_maps(hidden_states, c_attn_w, c_attn_b, c_proj_w)
    res = run_bass_kernel_spmd(nc, in_maps, list(range(N_CORES)))
    acc = res.results[0]["out"].astype(np.float32)
    for core in range(1, N_CORES):
        acc = acc + res.results[core]["out"]
    acc = acc + np.asarray(c_proj_b, dtype=np.float32)[None, :]
    return acc.reshape(B, S, D).astype(np.float32)


if __name__ == "__main__":
    rng = np.random.default_rng(0)
    hs = rng.standard_normal((B, S, D), dtype=np.float32)
    wa = rng.standard_normal((D, 3 * D), dtype=np.float32) * 0.02
    ba = rng.standard_normal((3 * D,), dtype=np.float32) * 0.02
    wp = rng.standard_normal((D, D), dtype=np.float32) * 0.02
    bp = rng.standard_normal((D,), dtype=np.float32) * 0.02
    out = kernel(hs, wa, ba, wp, bp)
    print("out", out.shape, out.dtype, float(np.abs(out).max()))


# revision 40
# speedup vs baseline: 1.2464x; 1.2464x over previous
"""GPT2-style fused causal attention (DecisionTransformer) on 8 Trainium2
NeuronCores — v4.

Sharding: tensor-parallel over the 16 heads (2 heads / core, both batches on
every core), row-parallel output projection; host sums the 8 partials.

vs v3 (188 us):
  - The exp conveyor starts at ~7 us instead of ~40 us: QKV runs as
    quarter-waves (1 PSUM bank, 512 tokens) and attention for (b0, qc0)
    begins as soon as the first q/k/v quarter + 4 vaug blocks exist.
    All remaining QKV waves, vaug builds and projections are emitted as
    filler units inside the attention kb loops.
  - V transposes packed: one [128,128] PE transpose per key block (both
    heads at once) instead of two [64,128] ones; vaug is a single
    [P, NKB, 2, P] tile per batch written with one strided DVE copy.
  - Causal masks run on the (otherwise idle) GpSimd engine.
  - Normalize: reciprocal on the [1, QC] denominator row BEFORE the
    partition broadcast (recip work drops 64x), broadcasts on GpSimd.
  - Evictions alternate between scalar/vector engines to balance load.
"""

import sys

for _p in ("/opt/trn_rl_repo",):
    if _p not in sys.path:
        sys.path.insert(0, _p)

import numpy as np
import ml_dtypes

import concourse.bass as bass
import concourse.mybir as mybir
import concourse.tile as tile
from concourse import bacc
from concourse.bass_utils import run_bass_kernel_spmd

P = 128
B, S, D, H, HD = 2, 2048, 1024, 16, 64
T = B * S              # 4096 tokens
KO = D // P            # 8 contraction chunks
QC = 512               # query chunk
NQC = S // QC          # 4
NKB = S // P           # 16 key blocks per sequence
SCALE = 1.0 / float(HD) ** 0.5
N_CORES = 8
HPC = H // N_CORES     # 2 heads per core

f32 = mybir.dt.float32
f32r = mybir.dt.float32r
bf16 = mybir.dt.bfloat16
BF = ml_dtypes.bfloat16


def _build_program():
    nc = bacc.Bacc(None, target_bir_lowering=False)

    xt_d = nc.dram_tensor("xt", [D, T], bf16, kind="ExternalInput")
    wqkv_d = nc.dram_tensor("w_qkv", [P, 3, KO * P], bf16, kind="ExternalInput")
    bqkv_d = nc.dram_tensor("b_qkv", [P, 3], f32, kind="ExternalInput")
    wp_d = nc.dram_tensor("w_proj", [P, D], bf16, kind="ExternalInput")
    # rank-127 factorization of the strict upper triangle: mu.T @ mv =
    # -1e5 * (k > q); accumulated onto diagonal score blocks in PSUM so
    # exp underflows masked entries to exactly 0 (no elementwise mask op)
    mu_d = nc.dram_tensor("mu", [P - 1, P], bf16, kind="ExternalInput")
    mv_d = nc.dram_tensor("mv", [P - 1, P], bf16, kind="ExternalInput")
    id128_d = nc.dram_tensor("id128", [P, P], bf16, kind="ExternalInput")
    out_d = nc.dram_tensor("out", [T, D], bf16, kind="ExternalOutput")

    with tile.TileContext(nc) as tc:
        with (
            tc.tile_pool(name="const", bufs=1) as const,
            tc.tile_pool(name="pt", bufs=5) as pt_pool,
            tc.tile_pool(name="atn", bufs=4) as atn_pool,
            tc.tile_pool(name="den", bufs=4) as den_pool,
            tc.tile_pool(name="rbs", bufs=4) as rbs_pool,
            tc.tile_pool(name="ot", bufs=3) as ot_pool,
            tc.tile_pool(name="ps_a", bufs=2, space="PSUM") as ps_a,
            tc.tile_pool(name="ps_sc", bufs=3, space="PSUM") as ps_sc,
            tc.tile_pool(name="ps_po", bufs=3, space="PSUM") as ps_po,
        ):
            # ---- constants (all host-prepared) ----
            bqkv_sb = const.tile([P, 3], f32)
            nc.sync.dma_start(bqkv_sb[:], bqkv_d[:])
            id128_sb = const.tile([P, P], bf16)
            nc.sync.dma_start(id128_sb[:], id128_d[:])
            # wqkv is fc-major [p, fc, ko, i]: per-fc slices are contiguous
            # so the first q columns land early in the DMA stream
            wqkv_sb = const.tile([P, 3, KO * P], bf16)
            nc.sync.dma_start(wqkv_sb[:, 0, :], wqkv_d[:, 0, :])
            mu_sb = const.tile([P - 1, P], bf16)
            nc.sync.dma_start(mu_sb[:], mu_d[:])
            mv_sb = const.tile([P - 1, P], bf16)
            nc.sync.dma_start(mv_sb[:], mv_d[:])
            wp_sb = const.tile([P, D], bf16)

            # persistent SBUF state (split per batch / per ko so the tile
            # dependency tracking stays fine-grained)
            xts = [
                [const.tile([P, S], bf16, name=f"xts{b}_{ko}") for ko in range(KO)]
                for b in range(B)
            ]
            # zero-padded Q^T per (batch, head): the other head's 64 rows
            # stay zero so full-128-contraction scores matmuls are exact
            qpad = [
                [const.tile([P, S], bf16, name=f"qp{b}{h}") for h in range(HPC)]
                for b in range(B)
            ]
            kT = [const.tile([P, S], bf16, name=f"kT{b}") for b in range(B)]
            vT = [const.tile([P, S], bf16, name=f"vT{b}") for b in range(B)]
            # V natural layout per batch: [keys, kb, head, 64 V | ones | 0pad]
            # (128-wide lhsT keeps the AV matmuls registering in the PE
            # clock gate; ones column yields the softmax denominator)
            vaug = [
                const.tile([P, NKB, HPC, P], bf16, name=f"vaug{b}")
                for b in range(B)
            ]
            # constant pad regions on the (idle-at-start) gpsimd engine so
            # neither the DMA rings nor the vector engine pay for them
            for b in range(B):
                nc.gpsimd.memset(qpad[b][0][HD:, :], 0.0)
                nc.gpsimd.memset(qpad[b][1][:HD, :], 0.0)
                nc.gpsimd.memset(vaug[b][:, :, :, HD : HD + 1], 1.0)
                nc.gpsimd.memset(vaug[b][:, :, :, HD + 1 :], 0.0)

            # HAM warmup: matmul activity while the first X chunks stream in
            for w in range(20):
                psw = ps_sc.tile([P, QC], f32, tag="sc", name="psw")
                nc.tensor.matmul(
                    psw[:, :P], id128_sb[:], id128_sb[:], start=True, stop=True
                )

            # X^T loads: batch 0 in 512-token x ko chunks so the first
            # quarter-wave completes after ~1 MB; batch 1 as full rows
            for ko in range(KO):
                nc.sync.dma_start(xts[0][ko][:, 0:QC],
                                  xt_d[ko * P : (ko + 1) * P, 0:QC])
            nc.sync.dma_start(wqkv_sb[:, 1, :], wqkv_d[:, 1, :])
            nc.sync.dma_start(wqkv_sb[:, 2, :], wqkv_d[:, 2, :])
            nc.sync.dma_start(wp_sb[:], wp_d[:])
            for tq in range(1, 4):
                for ko in range(KO):
                    nc.sync.dma_start(
                        xts[0][ko][:, tq * QC : (tq + 1) * QC],
                        xt_d[ko * P : (ko + 1) * P, tq * QC : (tq + 1) * QC],
                    )
            for ko in range(KO):
                nc.sync.dma_start(xts[1][ko][:],
                                  xt_d[ko * P : (ko + 1) * P, S : 2 * S])

            # ---------------- emit helpers ----------------
            ecnt = [0]

            def evict(dst_ap, src_ap, bias_ap=None, on_act=False):
                """PSUM->SBUF eviction. Once the exp conveyor is running,
                the scalar queue is deep — only route there when asked."""
                if bias_ap is None:
                    if on_act:
                        nc.scalar.copy(dst_ap, src_ap)
                    else:
                        nc.vector.tensor_copy(dst_ap, src_ap)
                elif on_act:
                    nc.scalar.activation(
                        dst_ap, src_ap,
                        mybir.ActivationFunctionType.Identity, bias=bias_ap,
                    )
                else:
                    nc.vector.tensor_scalar(
                        dst_ap, src_ap, bias_ap, None, mybir.AluOpType.add
                    )

            def qkv_half(b, fc, tq, half, state, on_act=False):
                """Half of a quarter wave: 4 ko-accumulation matmuls; the
                second half evicts. Splitting keeps PE filler bursts short
                so the score matmuls (and the exp conveyor) never starve."""
                if half == 0:
                    state["ps"] = ps_a.tile(
                        [P, QC], f32, tag="a", name=f"qkv{b}{fc}{tq}"
                    )
                ps = state["ps"]
                for ko in range(half * 4, half * 4 + 4):
                    nc.tensor.matmul(
                        ps[:],
                        wqkv_sb[:, fc, ko * P : (ko + 1) * P],
                        xts[b][ko][:, tq * QC : (tq + 1) * QC],
                        start=(ko == 0),
                        stop=(ko == KO - 1),
                    )
                if half == 0:
                    return
                cs = slice(tq * QC, (tq + 1) * QC)
                if fc == 0:
                    evict(qpad[b][0][:HD, cs], ps[:HD], bqkv_sb[:HD, 0:1],
                          on_act=on_act)
                    evict(qpad[b][1][HD:, cs], ps[HD:], bqkv_sb[HD:, 0:1],
                          on_act=on_act)
                else:
                    dst = kT[b] if fc == 1 else vT[b]
                    evict(dst[:, cs], ps[:], bqkv_sb[:, fc : fc + 1],
                          on_act=on_act)

            def qkv_wave(b, fc, tq, on_act=False):
                state = {}
                qkv_half(b, fc, tq, 0, state, on_act)
                qkv_half(b, fc, tq, 1, state, on_act)

            def vaug1(b, kb):
                """V natural layout for one key block: a single [128,128]
                PE transpose (both heads), one strided DVE copy."""
                psT = ps_a.tile([P, HPC, HD], bf16, tag="a", name=f"va{b}{kb}")
                nc.tensor.transpose(
                    psT[:], vT[b][:, kb * P : (kb + 1) * P], id128_sb[:]
                )
                nc.vector.tensor_copy(vaug[b][:, kb, :, :HD], psT[:])

            atn = [[None] * NQC for _ in range(B)]

            def attn_qc(b, qc, filler, draws=1):
                """Causal attention for both heads of batch b, query chunk
                qc; `draws` filler units are drained per key block."""
                nkb = (qc + 1) * (QC // P)
                po = [
                    ps_po.tile([P, QC], f32, tag="po", name=f"po{b}{qc}{h}")
                    for h in range(HPC)
                ]

                def av_pair(kb, pts, lo):
                    for hl in range(HPC):
                        nc.tensor.matmul(
                            po[hl][:, lo:],
                            vaug[b][:, kb, hl, :],
                            pts[hl][:, lo:],
                            start=(kb == 0),
                            stop=(kb == nkb - 1),
                        )

                prev = None
                for kb in range(nkb):
                    j = kb - qc * (QC // P)
                    lo = j * P if j > 0 else 0
                    pts = []
                    for hl in range(HPC):
                        sc = ps_sc.tile([P, QC], f32, tag="sc", name=f"sc{hl}")
                        nc.tensor.matmul(
                            sc[:, lo:],
                            kT[b][:, kb * P : (kb + 1) * P],
                            qpad[b][hl][:, qc * QC + lo : (qc + 1) * QC],
                            start=True,
                            stop=(j < 0),
                        )
                        if j >= 0:
                            # fold the causal mask into the score block:
                            # += mu.T @ mv = -1e5 above the diagonal
                            js = slice(j * P, (j + 1) * P)
                            nc.tensor.matmul(
                                sc[:, js], mu_sb[:], mv_sb[:],
                                start=False, stop=True,
                            )
                        pt = pt_pool.tile([P, QC], bf16, tag="pt", name=f"pt{hl}")
                        nc.scalar.activation(
                            pt[:, lo:], sc[:, lo:],
                            mybir.ActivationFunctionType.Exp, scale=SCALE,
                        )
                        pts.append(pt)
                    # AV lags the scores by one key block so the exp
                    # conveyor always has the next block queued
                    if prev is not None:
                        av_pair(*prev)
                    prev = (kb, pts, lo)
                    for _ in range(draws):
                        u = next(filler, None)
                        if u is not None:
                            u()
                av_pair(*prev)
                # normalize: den row -> reciprocal -> gpsimd broadcast ->
                # one DVE mult per head
                rbs = []
                for hl in range(HPC):
                    den = den_pool.tile([1, QC], f32, tag="den", name=f"dn{hl}")
                    if hl == 0:
                        nc.vector.tensor_copy(den[:], po[hl][HD : HD + 1, :])
                    else:
                        nc.scalar.copy(den[:], po[hl][HD : HD + 1, :])
                    rcp = den_pool.tile([1, QC], f32, tag="den", name=f"rc{hl}")
                    nc.vector.reciprocal_approx_fast(out=rcp[:], in_=den[:])
                    rb = rbs_pool.tile([HD, QC], f32, tag="rb", name=f"rb{hl}")
                    nc.gpsimd.partition_broadcast(rb[:], rcp[:], channels=HD)
                    rbs.append(rb)
                at = atn_pool.tile([P, QC], bf16, tag="atn", name=f"atn{b}{qc}")
                for hl in range(HPC):
                    hp = slice(hl * HD, (hl + 1) * HD)
                    nc.vector.tensor_tensor(
                        at[hp, :], po[hl][:HD, :], rbs[hl][:],
                        mybir.AluOpType.mult,
                    )
                atn[b][qc] = at

            def proj_unit(b, qc, qb, split_dma=False):
                ot = ot_pool.tile([P, D], bf16, tag="ot", name="ot")
                for nck in range(2):
                    pp = ps_a.tile(
                        [P, D // 2], f32, tag="a", name=f"pp{b}{qc}{qb}{nck}"
                    )
                    nc.tensor.matmul(
                        pp[:],
                        atn[b][qc][:, qb * P : (qb + 1) * P],
                        wp_sb[:, nck * (D // 2) : (nck + 1) * (D // 2)],
                        start=True,
                        stop=True,
                    )
                    on_act = ecnt[0] % 4 == 0
                    ecnt[0] += 1
                    evict(ot[:, nck * (D // 2) : (nck + 1) * (D // 2)], pp[:],
                          on_act=on_act)
                row = b * S + qc * QC + qb * P
                if split_dma:
                    nc.sync.dma_start(
                        out_d[row : row + P, : D // 2], ot[:, : D // 2]
                    )
                    nc.sync.dma_start(
                        out_d[row : row + P, D // 2 :], ot[:, D // 2 :]
                    )
                else:
                    nc.sync.dma_start(out_d[row : row + P, :], ot[:])

            # ---------------- schedule ----------------
            # prefix: first q/k/v quarters of batch 0 + vaug for kb 0-3
            # (scalar engine is still idle here — evict there)
            for fc in range(3):
                qkv_wave(0, fc, 0, on_act=True)
            for kb in range(4):
                vaug1(0, kb)

            def units():
                # (b, tq) quarter batches in need order: b0 tq1-3, b1 tq0-3.
                # Each is 6 wave half-units + 4 vaug units = 10 units; all 70
                # drain during b0 attention (2 units per key block).
                for b, tq in [(0, 1), (0, 2), (0, 3),
                              (1, 0), (1, 1), (1, 2), (1, 3)]:
                    for fc in range(3):
                        st = {}
                        yield lambda b=b, fc=fc, tq=tq, st=st: qkv_half(
                            b, fc, tq, 0, st)
                        yield lambda b=b, fc=fc, tq=tq, st=st: qkv_half(
                            b, fc, tq, 1, st)
                    for kb in range(4 * tq, 4 * tq + 4):
                        yield lambda b=b, kb=kb: vaug1(b, kb)
                # projections: consumed at the tail of b0 attention (draws
                # 71-80) and through b1 attention (1 per key block, draws
                # 81-120); bubbles delay proj(1,x) past attn(1,x)'s end
                for pb, pqc in [(0, 0), (0, 1), (0, 2), (0, 3)]:
                    for qb in range(QC // P):
                        yield lambda pb=pb, pqc=pqc, qb=qb: proj_unit(pb, pqc, qb)
                for qb in range(QC // P):          # draws 87-90 (attn(1,1))
                    yield lambda qb=qb: proj_unit(1, 0, qb)
                yield None
                yield None
                for qb in range(QC // P):          # draws 93-96 (attn(1,2))
                    yield lambda qb=qb: proj_unit(1, 1, qb)
                for _ in range(8):
                    yield None
                for qb in range(QC // P):          # draws 105-108 (attn(1,3))
                    yield lambda qb=qb: proj_unit(1, 2, qb)

            filler = units()
            for qc in range(NQC):
                attn_qc(0, qc, filler, draws=2)
            for qc in range(NQC):
                attn_qc(1, qc, filler, draws=1)
            # drain any unconsumed filler units (projections)
            for u in filler:
                if u is not None:
                    u()
            for qb in range(QC // P):
                proj_unit(1, 3, qb, split_dma=True)

    nc.compile()
    return nc


_CACHE = {}


def get_program():
    if "nc" not in _CACHE:
        _CACHE["nc"] = _build_program()
    return _CACHE["nc"]


def make_in_maps(hidden_states, c_attn_w, c_attn_b, c_proj_w):
    x = np.asarray(hidden_states, dtype=np.float32).reshape(T, D)
    xt = np.ascontiguousarray(x.T).astype(BF)                     # [D, T]
    wa = np.asarray(c_attn_w, dtype=np.float32)
    ba = np.asarray(c_attn_b, dtype=np.float32)
    wp = np.asarray(c_proj_w, dtype=np.float32)

    # mu.T @ mv = -1e5 * strict_upper_triangle(P): mu[c,k] = (k > c),
    # mv[c,q] = -1e5 * (q == c)
    cc, kk = np.meshgrid(np.arange(P - 1), np.arange(P), indexing="ij")
    mu = (kk > cc).astype(BF)                                     # [P-1, P]
    mv = np.zeros((P - 1, P), dtype=np.float32)
    mv[np.arange(P - 1), np.arange(P - 1)] = -1e5
    mv = mv.astype(BF)
    id128 = np.eye(P, dtype=np.float32).astype(BF)                # [P, P]

    in_maps = []
    for core in range(N_CORES):
        lo = core * P
        # [d, fc, i] -> [p, ko, fc, i] -> [P, KO*3*P]
        wa3 = np.stack(
            [wa[:, lo : lo + P], wa[:, D + lo : D + lo + P],
             wa[:, 2 * D + lo : 2 * D + lo + P]],
            axis=1,
        )                                                          # [D, 3, P]
        # fc-major: [p, fc, ko, i]
        wq = np.ascontiguousarray(
            wa3.reshape(KO, P, 3, P).transpose(1, 2, 0, 3).reshape(P, 3, KO * P)
        ).astype(BF)
        bq = np.ascontiguousarray(
            np.stack(
                [ba[lo : lo + P], ba[D + lo : D + lo + P],
                 ba[2 * D + lo : 2 * D + lo + P]],
                axis=1,
            )
        ).astype(np.float32)                                       # [P, 3]
        wpc = np.ascontiguousarray(wp[lo : lo + P, :]).astype(BF)  # [P, D]
        in_maps.append(
            {
                "xt": xt,
                "w_qkv": wq,
                "b_qkv": bq,
                "w_proj": wpc,
                "mu": mu,
                "mv": mv,
                "id128": id128,
            }
        )
    return in_maps


def kernel(hidden_states, c_attn_w, c_attn_b, c_proj_w, c_proj_b):
    nc = get_program()
    in_maps = make_in_maps(hidden_states, c_attn_w, c_attn_b, c_proj_w)
    res = run_bass_kernel_spmd(nc, in_maps, list(range(N_CORES)))
    acc = res.results[0]["out"].astype(np.float32)
    for core in range(1, N_CORES):
        acc = acc + res.results[core]["out"]
    acc = acc + np.asarray(c_proj_b, dtype=np.float32)[None, :]
    return acc.reshape(B, S, D).astype(np.float32)


if __name__ == "__main__":
    rng = np.random.default_rng(0)
    hs = rng.standard_normal((B, S, D), dtype=np.float32)
    wa = rng.standard_normal((D, 3 * D), dtype=np.float32) * 0.02
    ba = rng.standard_normal((3 * D,), dtype=np.float32) * 0.02
    wp = rng.standard_normal((D, D), dtype=np.float32) * 0.02
    bp = rng.standard_normal((D,), dtype=np.float32) * 0.02
    out = kernel(hs, wa, ba, wp, bp)
    print("out", out.shape, out.dtype, float(np.abs(out).max()))


# revision 41
# speedup vs baseline: 1.2593x; 1.0103x over previous
"""GPT2-style fused causal attention (DecisionTransformer) on 8 Trainium2
NeuronCores — v4.

Sharding: tensor-parallel over the 16 heads (2 heads / core, both batches on
every core), row-parallel output projection; host sums the 8 partials.

vs v3 (188 us):
  - The exp conveyor starts at ~7 us instead of ~40 us: QKV runs as
    quarter-waves (1 PSUM bank, 512 tokens) and attention for (b0, qc0)
    begins as soon as the first q/k/v quarter + 4 vaug blocks exist.
    All remaining QKV waves, vaug builds and projections are emitted as
    filler units inside the attention kb loops.
  - V transposes packed: one [128,128] PE transpose per key block (both
    heads at once) instead of two [64,128] ones; vaug is a single
    [P, NKB, 2, P] tile per batch written with one strided DVE copy.
  - Causal masks run on the (otherwise idle) GpSimd engine.
  - Normalize: reciprocal on the [1, QC] denominator row BEFORE the
    partition broadcast (recip work drops 64x), broadcasts on GpSimd.
  - Evictions alternate between scalar/vector engines to balance load.
"""

import sys

for _p in ("/opt/trn_rl_repo",):
    if _p not in sys.path:
        sys.path.insert(0, _p)

import numpy as np
import ml_dtypes

import concourse.bass as bass
import concourse.mybir as mybir
import concourse.tile as tile
from concourse import bacc
from concourse.bass_utils import run_bass_kernel_spmd

P = 128
B, S, D, H, HD = 2, 2048, 1024, 16, 64
T = B * S              # 4096 tokens
KO = D // P            # 8 contraction chunks
QC = 512               # query chunk
NQC = S // QC          # 4
NKB = S // P           # 16 key blocks per sequence
SCALE = 1.0 / float(HD) ** 0.5
N_CORES = 8
HPC = H // N_CORES     # 2 heads per core

f32 = mybir.dt.float32
f32r = mybir.dt.float32r
bf16 = mybir.dt.bfloat16
BF = ml_dtypes.bfloat16


def _build_program():
    nc = bacc.Bacc(None, target_bir_lowering=False)

    xt_d = nc.dram_tensor("xt", [D, T], bf16, kind="ExternalInput")
    wqkv_d = nc.dram_tensor("w_qkv", [P, 3, KO * P], bf16, kind="ExternalInput")
    bqkv_d = nc.dram_tensor("b_qkv", [P, 3], f32, kind="ExternalInput")
    wp_d = nc.dram_tensor("w_proj", [P, D], bf16, kind="ExternalInput")
    # rank-127 factorization of the strict upper triangle: mu.T @ mv =
    # -1e5 * (k > q); accumulated onto diagonal score blocks in PSUM so
    # exp underflows masked entries to exactly 0 (no elementwise mask op)
    mu_d = nc.dram_tensor("mu", [P - 1, P], bf16, kind="ExternalInput")
    mv_d = nc.dram_tensor("mv", [P - 1, P], bf16, kind="ExternalInput")
    id128_d = nc.dram_tensor("id128", [P, P], bf16, kind="ExternalInput")
    out_d = nc.dram_tensor("out", [T, D], bf16, kind="ExternalOutput")

    with tile.TileContext(nc) as tc:
        with (
            tc.tile_pool(name="const", bufs=1) as const,
            tc.tile_pool(name="pt", bufs=5) as pt_pool,
            tc.tile_pool(name="atn", bufs=4) as atn_pool,
            tc.tile_pool(name="den", bufs=4) as den_pool,
            tc.tile_pool(name="rbs", bufs=4) as rbs_pool,
            tc.tile_pool(name="ot", bufs=3) as ot_pool,
            tc.tile_pool(name="ps_a", bufs=2, space="PSUM") as ps_a,
            tc.tile_pool(name="ps_sc", bufs=3, space="PSUM") as ps_sc,
            tc.tile_pool(name="ps_po", bufs=3, space="PSUM") as ps_po,
        ):
            # ---- constants (all host-prepared) ----
            bqkv_sb = const.tile([P, 3], f32)
            nc.sync.dma_start(bqkv_sb[:], bqkv_d[:])
            id128_sb = const.tile([P, P], bf16)
            nc.sync.dma_start(id128_sb[:], id128_d[:])
            # wqkv is fc-major [p, fc, ko, i]: per-fc slices are contiguous
            # so the first q columns land early in the DMA stream
            wqkv_sb = const.tile([P, 3, KO * P], bf16)
            nc.sync.dma_start(wqkv_sb[:, 0, :], wqkv_d[:, 0, :])
            mu_sb = const.tile([P - 1, P], bf16)
            nc.sync.dma_start(mu_sb[:], mu_d[:])
            mv_sb = const.tile([P - 1, P], bf16)
            nc.sync.dma_start(mv_sb[:], mv_d[:])
            wp_sb = const.tile([P, D], bf16)

            # persistent SBUF state (split per batch / per ko so the tile
            # dependency tracking stays fine-grained)
            xts = [
                [const.tile([P, S], bf16, name=f"xts{b}_{ko}") for ko in range(KO)]
                for b in range(B)
            ]
            # zero-padded Q^T per (batch, head): the other head's 64 rows
            # stay zero so full-128-contraction scores matmuls are exact
            qpad = [
                [const.tile([P, S], bf16, name=f"qp{b}{h}") for h in range(HPC)]
                for b in range(B)
            ]
            kT = [const.tile([P, S], bf16, name=f"kT{b}") for b in range(B)]
            vT = [const.tile([P, S], bf16, name=f"vT{b}") for b in range(B)]
            # V natural layout per batch: [keys, kb, head, 64 V | ones | 0pad]
            # (128-wide lhsT keeps the AV matmuls registering in the PE
            # clock gate; ones column yields the softmax denominator)
            vaug = [
                const.tile([P, NKB, HPC, P], bf16, name=f"vaug{b}")
                for b in range(B)
            ]
            # constant pad regions on the (idle-at-start) gpsimd engine so
            # neither the DMA rings nor the vector engine pay for them
            for b in range(B):
                nc.gpsimd.memset(qpad[b][0][HD:, :], 0.0)
                nc.gpsimd.memset(qpad[b][1][:HD, :], 0.0)
                nc.gpsimd.memset(vaug[b][:, :, :, HD : HD + 1], 1.0)
                nc.gpsimd.memset(vaug[b][:, :, :, HD + 1 :], 0.0)

            # HAM warmup: matmul activity while the first X chunks stream in
            for w in range(20):
                psw = ps_sc.tile([P, QC], f32, tag="sc", name="psw")
                nc.tensor.matmul(
                    psw[:, :P], id128_sb[:], id128_sb[:], start=True, stop=True
                )

            # X^T loads: batch 0 in 512-token x ko chunks so the first
            # quarter-wave completes after ~1 MB; batch 1 as full rows
            for ko in range(KO):
                nc.sync.dma_start(xts[0][ko][:, 0:QC],
                                  xt_d[ko * P : (ko + 1) * P, 0:QC])
            nc.sync.dma_start(wqkv_sb[:, 1, :], wqkv_d[:, 1, :])
            nc.sync.dma_start(wqkv_sb[:, 2, :], wqkv_d[:, 2, :])
            nc.sync.dma_start(wp_sb[:], wp_d[:])
            for tq in range(1, 4):
                for ko in range(KO):
                    nc.sync.dma_start(
                        xts[0][ko][:, tq * QC : (tq + 1) * QC],
                        xt_d[ko * P : (ko + 1) * P, tq * QC : (tq + 1) * QC],
                    )
            for ko in range(KO):
                nc.sync.dma_start(xts[1][ko][:],
                                  xt_d[ko * P : (ko + 1) * P, S : 2 * S])

            # ---------------- emit helpers ----------------
            ecnt = [0]

            def evict(dst_ap, src_ap, bias_ap=None, on_act=False):
                """PSUM->SBUF eviction. Once the exp conveyor is running,
                the scalar queue is deep — only route there when asked."""
                if bias_ap is None:
                    if on_act:
                        nc.scalar.copy(dst_ap, src_ap)
                    else:
                        nc.vector.tensor_copy(dst_ap, src_ap)
                elif on_act:
                    nc.scalar.activation(
                        dst_ap, src_ap,
                        mybir.ActivationFunctionType.Identity, bias=bias_ap,
                    )
                else:
                    nc.vector.tensor_scalar(
                        dst_ap, src_ap, bias_ap, None, mybir.AluOpType.add
                    )

            def qkv_half(b, fc, tq, half, state, on_act=False):
                """Half of a quarter wave: 4 ko-accumulation matmuls; the
                second half evicts. Splitting keeps PE filler bursts short
                so the score matmuls (and the exp conveyor) never starve."""
                if half == 0:
                    state["ps"] = ps_a.tile(
                        [P, QC], f32, tag="a", name=f"qkv{b}{fc}{tq}"
                    )
                ps = state["ps"]
                for ko in range(half * 4, half * 4 + 4):
                    nc.tensor.matmul(
                        ps[:],
                        wqkv_sb[:, fc, ko * P : (ko + 1) * P],
                        xts[b][ko][:, tq * QC : (tq + 1) * QC],
                        start=(ko == 0),
                        stop=(ko == KO - 1),
                    )
                if half == 0:
                    return
                cs = slice(tq * QC, (tq + 1) * QC)
                if fc == 0:
                    evict(qpad[b][0][:HD, cs], ps[:HD], bqkv_sb[:HD, 0:1],
                          on_act=on_act)
                    evict(qpad[b][1][HD:, cs], ps[HD:], bqkv_sb[HD:, 0:1],
                          on_act=on_act)
                else:
                    dst = kT[b] if fc == 1 else vT[b]
                    evict(dst[:, cs], ps[:], bqkv_sb[:, fc : fc + 1],
                          on_act=on_act)

            def qkv_wave(b, fc, tq, on_act=False):
                state = {}
                qkv_half(b, fc, tq, 0, state, on_act)
                qkv_half(b, fc, tq, 1, state, on_act)

            def vaug1(b, kb):
                """V natural layout for one key block: a single [128,128]
                PE transpose (both heads), one strided DVE copy."""
                psT = ps_a.tile([P, HPC, HD], bf16, tag="a", name=f"va{b}{kb}")
                nc.tensor.transpose(
                    psT[:], vT[b][:, kb * P : (kb + 1) * P], id128_sb[:]
                )
                nc.vector.tensor_copy(vaug[b][:, kb, :, :HD], psT[:])

            atn = [[None] * NQC for _ in range(B)]

            def attn_qc(b, qc, filler, draws=1):
                """Causal attention for both heads of batch b, query chunk
                qc; `draws` filler units are drained per key block."""
                nkb = (qc + 1) * (QC // P)
                po = [
                    ps_po.tile([P, QC], f32, tag="po", name=f"po{b}{qc}{h}")
                    for h in range(HPC)
                ]

                def av_pair(kb, pts, lo):
                    for hl in range(HPC):
                        nc.tensor.matmul(
                            po[hl][:, lo:],
                            vaug[b][:, kb, hl, :],
                            pts[hl][:, lo:],
                            start=(kb == 0),
                            stop=(kb == nkb - 1),
                        )

                prev = None
                for kb in range(nkb):
                    j = kb - qc * (QC // P)
                    lo = j * P if j > 0 else 0
                    pts = []
                    for hl in range(HPC):
                        sc = ps_sc.tile([P, QC], f32, tag="sc", name=f"sc{hl}")
                        nc.tensor.matmul(
                            sc[:, lo:],
                            kT[b][:, kb * P : (kb + 1) * P],
                            qpad[b][hl][:, qc * QC + lo : (qc + 1) * QC],
                            start=True,
                            stop=(j < 0),
                        )
                        if j >= 0:
                            # fold the causal mask into the score block:
                            # += mu.T @ mv = -1e5 above the diagonal
                            js = slice(j * P, (j + 1) * P)
                            nc.tensor.matmul(
                                sc[:, js], mu_sb[:], mv_sb[:],
                                start=False, stop=True,
                            )
                        pt = pt_pool.tile([P, QC], bf16, tag="pt", name=f"pt{hl}")
                        nc.scalar.activation(
                            pt[:, lo:], sc[:, lo:],
                            mybir.ActivationFunctionType.Exp, scale=SCALE,
                        )
                        pts.append(pt)
                    # AV lags the scores by one key block so the exp
                    # conveyor always has the next block queued
                    if prev is not None:
                        av_pair(*prev)
                    prev = (kb, pts, lo)
                    for _ in range(draws):
                        u = next(filler, None)
                        if u is not None:
                            u()
                av_pair(*prev)
                # normalize: den row -> reciprocal -> gpsimd broadcast ->
                # one DVE mult per head. High priority: the sooner po is
                # read out, the sooner its banks recycle for the next chunk.
                with tc.high_priority(offset=200):
                    rbs = []
                    for hl in range(HPC):
                        den = den_pool.tile([1, QC], f32, tag="den",
                                            name=f"dn{hl}")
                        if hl == 0:
                            nc.vector.tensor_copy(den[:], po[hl][HD : HD + 1, :])
                        else:
                            nc.scalar.copy(den[:], po[hl][HD : HD + 1, :])
                        rcp = den_pool.tile([1, QC], f32, tag="den",
                                            name=f"rc{hl}")
                        nc.vector.reciprocal_approx_fast(out=rcp[:], in_=den[:])
                        rb = rbs_pool.tile([HD, QC], f32, tag="rb",
                                           name=f"rb{hl}")
                        nc.gpsimd.partition_broadcast(rb[:], rcp[:], channels=HD)
                        rbs.append(rb)
                    at = atn_pool.tile([P, QC], bf16, tag="atn",
                                       name=f"atn{b}{qc}")
                    for hl in range(HPC):
                        hp = slice(hl * HD, (hl + 1) * HD)
                        nc.vector.tensor_tensor(
                            at[hp, :], po[hl][:HD, :], rbs[hl][:],
                            mybir.AluOpType.mult,
                        )
                    atn[b][qc] = at

            def proj_unit(b, qc, qb, split_dma=False):
                ot = ot_pool.tile([P, D], bf16, tag="ot", name="ot")
                for nck in range(2):
                    pp = ps_a.tile(
                        [P, D // 2], f32, tag="a", name=f"pp{b}{qc}{qb}{nck}"
                    )
                    nc.tensor.matmul(
                        pp[:],
                        atn[b][qc][:, qb * P : (qb + 1) * P],
                        wp_sb[:, nck * (D // 2) : (nck + 1) * (D // 2)],
                        start=True,
                        stop=True,
                    )
                    on_act = ecnt[0] % 4 == 0
                    ecnt[0] += 1
                    evict(ot[:, nck * (D // 2) : (nck + 1) * (D // 2)], pp[:],
                          on_act=on_act)
                row = b * S + qc * QC + qb * P
                if split_dma:
                    nc.sync.dma_start(
                        out_d[row : row + P, : D // 2], ot[:, : D // 2]
                    )
                    nc.sync.dma_start(
                        out_d[row : row + P, D // 2 :], ot[:, D // 2 :]
                    )
                else:
                    nc.sync.dma_start(out_d[row : row + P, :], ot[:])

            # ---------------- schedule ----------------
            # prefix: first q/k/v quarters of batch 0 + vaug for kb 0-3
            # (scalar engine is still idle here — evict there)
            for fc in range(3):
                qkv_wave(0, fc, 0, on_act=True)
            for kb in range(4):
                vaug1(0, kb)

            def units():
                # (b, tq) quarter batches in need order: b0 tq1-3, b1 tq0-3.
                # Each is 6 wave half-units + 4 vaug units = 10 units; all 70
                # drain during b0 attention (2 units per key block).
                for b, tq in [(0, 1), (0, 2), (0, 3),
                              (1, 0), (1, 1), (1, 2), (1, 3)]:
                    for fc in range(3):
                        st = {}
                        yield lambda b=b, fc=fc, tq=tq, st=st: qkv_half(
                            b, fc, tq, 0, st)
                        yield lambda b=b, fc=fc, tq=tq, st=st: qkv_half(
                            b, fc, tq, 1, st)
                    for kb in range(4 * tq, 4 * tq + 4):
                        yield lambda b=b, kb=kb: vaug1(b, kb)
                # projections: consumed at the tail of b0 attention (draws
                # 71-80) and through b1 attention (1 per key block, draws
                # 81-120); bubbles delay proj(1,x) past attn(1,x)'s end
                for pb, pqc in [(0, 0), (0, 1), (0, 2), (0, 3)]:
                    for qb in range(QC // P):
                        yield lambda pb=pb, pqc=pqc, qb=qb: proj_unit(pb, pqc, qb)
                for qb in range(QC // P):          # draws 87-90 (attn(1,1))
                    yield lambda qb=qb: proj_unit(1, 0, qb)
                yield None
                yield None
                for qb in range(QC // P):          # draws 93-96 (attn(1,2))
                    yield lambda qb=qb: proj_unit(1, 1, qb)
                for _ in range(8):
                    yield None
                for qb in range(QC // P):          # draws 105-108 (attn(1,3))
                    yield lambda qb=qb: proj_unit(1, 2, qb)

            filler = units()
            for qc in range(NQC):
                attn_qc(0, qc, filler, draws=2)
            for qc in range(NQC):
                attn_qc(1, qc, filler, draws=1)
            # drain any unconsumed filler units (projections)
            for u in filler:
                if u is not None:
                    u()
            # fast tail: per-qb normalize+proj pipeline for the last chunk
            at = atn[1][3]
            for qb in range(QC // P):
                qs = slice(qb * P, (qb + 1) * P)
                ot = ot_pool.tile([P, D], bf16, tag="ot", name="ot")
                for nck in range(2):
                    pp = ps_a.tile([P, D // 2], f32, tag="a", name=f"fpp{qb}{nck}")
                    nc.tensor.matmul(
                        pp[:],
                        at[:, qs],
                        wp_sb[:, nck * (D // 2) : (nck + 1) * (D // 2)],
                        start=True,
                        stop=True,
                    )
                    evict(ot[:, nck * (D // 2) : (nck + 1) * (D // 2)], pp[:],
                          on_act=(nck == 0))
                row = S + 3 * QC + qb * P
                nc.sync.dma_start(out_d[row : row + P, : D // 2],
                                  ot[:, : D // 2])
                nc.sync.dma_start(out_d[row : row + P, D // 2 :],
                                  ot[:, D // 2 :])

    nc.compile()
    return nc


_CACHE = {}


def get_program():
    if "nc" not in _CACHE:
        _CACHE["nc"] = _build_program()
    return _CACHE["nc"]


def make_in_maps(hidden_states, c_attn_w, c_attn_b, c_proj_w):
    x = np.asarray(hidden_states, dtype=np.float32).reshape(T, D)
    xt = np.ascontiguousarray(x.T).astype(BF)                     # [D, T]
    wa = np.asarray(c_attn_w, dtype=np.float32)
    ba = np.asarray(c_attn_b, dtype=np.float32)
    wp = np.asarray(c_proj_w, dtype=np.float32)

    # mu.T @ mv = -1e5 * strict_upper_triangle(P): mu[c,k] = (k > c),
    # mv[c,q] = -1e5 * (q == c)
    cc, kk = np.meshgrid(np.arange(P - 1), np.arange(P), indexing="ij")
    mu = (kk > cc).astype(BF)                                     # [P-1, P]
    mv = np.zeros((P - 1, P), dtype=np.float32)
    mv[np.arange(P - 1), np.arange(P - 1)] = -1e5
    mv = mv.astype(BF)
    id128 = np.eye(P, dtype=np.float32).astype(BF)                # [P, P]

    in_maps = []
    for core in range(N_CORES):
        lo = core * P
        # [d, fc, i] -> [p, ko, fc, i] -> [P, KO*3*P]
        wa3 = np.stack(
            [wa[:, lo : lo + P], wa[:, D + lo : D + lo + P],
             wa[:, 2 * D + lo : 2 * D + lo + P]],
            axis=1,
        )                                                          # [D, 3, P]
        # fc-major: [p, fc, ko, i]
        wq = np.ascontiguousarray(
            wa3.reshape(KO, P, 3, P).transpose(1, 2, 0, 3).reshape(P, 3, KO * P)
        ).astype(BF)
        bq = np.ascontiguousarray(
            np.stack(
                [ba[lo : lo + P], ba[D + lo : D + lo + P],
                 ba[2 * D + lo : 2 * D + lo + P]],
                axis=1,
            )
        ).astype(np.float32)                                       # [P, 3]
        wpc = np.ascontiguousarray(wp[lo : lo + P, :]).astype(BF)  # [P, D]
        in_maps.append(
            {
                "xt": xt,
                "w_qkv": wq,
                "b_qkv": bq,
                "w_proj": wpc,
                "mu": mu,
                "mv": mv,
                "id128": id128,
            }
        )
    return in_maps


def kernel(hidden_states, c_attn_w, c_attn_b, c_proj_w, c_proj_b):
    nc = get_program()
    in_maps = make_in_maps(hidden_states, c_attn_w, c_attn_b, c_proj_w)
    res = run_bass_kernel_spmd(nc, in_maps, list(range(N_CORES)))
    acc = res.results[0]["out"].astype(np.float32)
    for core in range(1, N_CORES):
        acc = acc + res.results[core]["out"]
    acc = acc + np.asarray(c_proj_b, dtype=np.float32)[None, :]
    return acc.reshape(B, S, D).astype(np.float32)


if __name__ == "__main__":
    rng = np.random.default_rng(0)
    hs = rng.standard_normal((B, S, D), dtype=np.float32)
    wa = rng.standard_normal((D, 3 * D), dtype=np.float32) * 0.02
    ba = rng.standard_normal((3 * D,), dtype=np.float32) * 0.02
    wp = rng.standard_normal((D, D), dtype=np.float32) * 0.02
    bp = rng.standard_normal((D,), dtype=np.float32) * 0.02
    out = kernel(hs, wa, ba, wp, bp)
    print("out", out.shape, out.dtype, float(np.abs(out).max()))
